# Initial kernel scaffold
#
"""Trainium2 Bass kernel for nn_MultiHeadAttention (B=2, S=2048, D=1024, H=16).

Sharding: batch x head-groups across 8 cores. Core c handles batch c//4 and
heads 4*(c%4) .. 4*(c%4)+3 (4 "units" per core). Each core computes, per unit:
  - projections qhT/khT (f32r, head-dim on partitions) and vh (bf16, keys on
    partitions) for its 4 heads
  - a k-major pass: scoresT = kh^T q (f32r) + mask bias (fp8 identity-matmul
    add), exp on ScalarE -> probsT (bf16, unnormalized), AV matmul with a
    ones-column appended to vh so row 64 of the PSUM accumulator is the
    per-query masked rowsum; att_out is normalized with 1/rowsum broadcast
  - a q-major pass: scores = qh^T k + mask bias, exp with accum_out giving the
    masked rowsum per query row, normalize on VectorE, DMA the fp32 attention
    probabilities straight out
  - an output projection accumulating all 4 heads in PSUM -> partial out
Host side only reshapes/slices inputs, and gathers: attn slabs are stacked,
partial outs summed across the 4 cores of each batch.
"""

import numpy as np
import ml_dtypes
from contextlib import ExitStack

import concourse.bass as bass
import concourse.tile as tile
from concourse import bacc, mybir
from concourse.bass_utils import run_bass_kernel_spmd
from concourse.masks import make_identity

B, S, D, H = 2, 2048, 1024, 16
DH = D // H          # 64
NCORES = 8
HPC = 4              # heads per core
SB = 512             # matmul moving-dim block
NQT = S // 128       # 16 q/k tiles per unit
NDC = D // 128       # 8 contraction chunks
MASK_BIAS = -32.0    # exp(-32+s) ~ 1e-14: numerically zero vs rowsum ~1e3

F32 = mybir.dt.float32
F32R = mybir.dt.float32r
BF16 = mybir.dt.bfloat16
FP8 = mybir.dt.float8e5


def build_nc(reps: int = 1):
    """Build + compile the per-core Bass kernel. reps>1 repeats the whole
    compute (same outputs) for wall-clock benchmarking."""
    nc = bacc.Bacc("TRN2", target_bir_lowering=False, debug=False)

    def din(name, shape, dt):
        return nc.dram_tensor(name, shape, dt, kind="ExternalInput").ap()

    def dout(name, shape, dt):
        return nc.dram_tensor(name, shape, dt, kind="ExternalOutput").ap()

    qT_d = din("qT", [D, S], F32)
    kT_d = din("kT", [D, S], F32)
    vT_d = din("vT", [D, S], F32)
    wq_d = din("wq", [D, 2, 128], F32)      # [d, pair, 2*64] head-pair lhsT cols
    wk_d = din("wk", [D, 2, 128], F32)
    wv_d = din("wv", [D, HPC * DH], F32)    # [d, 4*64]
    wo_d = din("wo", [64, HPC, D], F32)     # [dh, unit, dout]
    mq_d = din("mq8", [S, S], FP8)          # q-major mask bias (0 / -32)
    mt_d = din("mt8", [S, S], FP8)          # k-major (transposed) mask bias
    attn_d = dout("attn_part", [HPC, S, S], F32)
    out_d = dout("out_part", [S, D], F32)

    with tile.TileContext(nc) as tc, ExitStack() as ctx:
        res = ctx.enter_context(tc.tile_pool(name="res", bufs=1))
        xin = ctx.enter_context(tc.tile_pool(name="xin", bufs=3))
        xr = ctx.enter_context(tc.tile_pool(name="xr", bufs=3))
        work = ctx.enter_context(tc.tile_pool(name="work", bufs=2))
        ps_sc = ctx.enter_context(tc.tile_pool(name="ps_sc", bufs=2, space="PSUM"))
        ps_av = ctx.enter_context(tc.tile_pool(name="ps_av", bufs=1, space="PSUM"))

        # ---- resident tensors ----
        mq_sb = res.tile([128, NQT, S], FP8)     # [p, qtile, k]
        nc.sync.dma_start(mq_sb[:], mq_d.rearrange("(t p) k -> p t k", p=128))
        mt_sb = res.tile([128, NQT, S], FP8)     # [p, ktile, q]
        nc.sync.dma_start(mt_sb[:], mt_d.rearrange("(t p) q -> p t q", p=128))

        id_f = res.tile([128, 128], F32)
        make_identity(nc, id_f[:])
        id8 = res.tile([128, 128], FP8)
        nc.vector.tensor_copy(id8[:], id_f[:])

        def load_round(dram_ap, shape, dt, name):
            t_f = res.tile(shape, F32, name=name + "_f")
            nc.sync.dma_start(t_f[:], dram_ap)
            t_r = res.tile(shape, dt, name=name + "_r")
            nc.vector.tensor_copy(t_r[:], t_f[:])
            return t_r

        wq_sb = load_round(wq_d.rearrange("(c p) j m -> p c j m", p=128),
                           [128, NDC, 2, 128], F32R, "wq")
        wk_sb = load_round(wk_d.rearrange("(c p) j m -> p c j m", p=128),
                           [128, NDC, 2, 128], F32R, "wk")
        wv_sb = load_round(wv_d.rearrange("(c p) m -> p c m", p=128),
                           [128, NDC, HPC * DH], BF16, "wv")
        wo_sb = load_round(wo_d, [64, HPC, D], F32R, "wo")

        qhT = res.tile([128, 2, S], F32R)        # [64*half+dh, pair, s]
        khT = res.tile([128, 2, S], F32R)
        vh = res.tile([128, NQT, HPC * 65], BF16)  # [key%128, kchunk, 65h+dh|ones]
        ao = res.tile([64, HPC, S], F32R)        # normalized att_out^T per unit

        ones_f = res.tile([128, NQT, HPC], F32)
        nc.gpsimd.memset(ones_f[:], 1.0)
        nc.vector.tensor_copy(
            vh[:].rearrange("p c (h x) -> p c h x", x=65)[:, :, :, 64], ones_f[:])

        for _ in range(reps):
            # ---- projections: qhT / khT ----
            for src_d, wsb, dst in ((qT_d, wq_sb, qhT), (kT_d, wk_sb, khT)):
                for sb_i in range(S // SB):
                    pss = []
                    for pair in range(2):
                        ps = ps_sc.tile([128, 1024], F32, tag="ps_sc",
                                        name=f"pspr{pair}")
                        pss.append(ps)
                    for c in range(NDC):
                        x_f = xin.tile([128, SB], F32, tag="x_f", name="x_f")
                        nc.sync.dma_start(
                            x_f[:], src_d[c * 128:(c + 1) * 128,
                                          sb_i * SB:(sb_i + 1) * SB])
                        x_r = xr.tile([128, SB], F32R, tag="x_r", name="x_r")
                        nc.vector.tensor_copy(x_r[:], x_f[:])
                        for pair in range(2):
                            nc.tensor.matmul(pss[pair][:, :SB], wsb[:, c, pair, :],
                                             x_r[:], start=(c == 0),
                                             stop=(c == NDC - 1))
                    for pair in range(2):
                        nc.vector.tensor_copy(
                            dst[:, pair, sb_i * SB:(sb_i + 1) * SB],
                            pss[pair][:, :SB])

            # ---- projections: vh (keys on partitions, bf16) ----
            for sb_i in range(S // SB):
                pss = []
                for j in range(SB // 128):
                    ps = ps_sc.tile([128, 1024], F32, tag="ps_sc", name=f"psv{j}")
                    pss.append(ps)
                for c in range(NDC):
                    x_f = xin.tile([128, SB], F32, tag="x_f", name="xv_f")
                    nc.sync.dma_start(
                        x_f[:], vT_d[c * 128:(c + 1) * 128,
                                     sb_i * SB:(sb_i + 1) * SB])
                    x_b = xr.tile([128, SB], BF16, tag="x_b", name="xv_b")
                    nc.vector.tensor_copy(x_b[:], x_f[:])
                    for j in range(SB // 128):
                        nc.tensor.matmul(pss[j][:, :HPC * DH],
                                         x_b[:, j * 128:(j + 1) * 128],
                                         wv_sb[:, c, :], start=(c == 0),
                                         stop=(c == NDC - 1))
                for j in range(SB // 128):
                    kc = sb_i * (SB // 128) + j
                    nc.vector.tensor_copy(
                        vh[:, kc, :].rearrange("p (h x) -> p h x", x=65)[:, :, :64],
                        pss[j][:, :HPC * DH].rearrange("p (h x) -> p h x", x=64))

            # ---- per-unit attention ----
            for u in range(HPC):
                half, pair = u % 2, u // 2
                qh_u = qhT[64 * half:64 * half + 64, pair, :]
                kh_u = khT[64 * half:64 * half + 64, pair, :]

                # K-major pass: AV + rowsums
                av_ps = ps_av.tile([65, S], F32, tag="ps_av", name="av_ps")
                for kc in range(NQT):
                    sc_ps = []
                    for hk in range(2):
                        ps = ps_sc.tile([128, 1024], F32, tag="ps_sc",
                                        name=f"sct{hk}")
                        for qb in range(2):
                            sl = slice(qb * SB, (qb + 1) * SB)
                            qoff = hk * 1024
                            nc.tensor.matmul(
                                ps[:, sl],
                                kh_u[:, kc * 128:(kc + 1) * 128],
                                qh_u[:, qoff + qb * SB:qoff + (qb + 1) * SB],
                                start=True, stop=False)
                            nc.tensor.matmul(
                                ps[:, sl], id8[:],
                                mt_sb[:, kc, qoff + qb * SB:qoff + (qb + 1) * SB],
                                start=False, stop=True)
                        sc_ps.append(ps)
                    expT = work.tile([128, S], BF16, tag="expT", name="expT")
                    for hk in range(2):
                        nc.scalar.activation(expT[:, hk * 1024:(hk + 1) * 1024],
                                             sc_ps[hk][:],
                                             mybir.ActivationFunctionType.Exp)
                    for qb4 in range(S // SB):
                        sl = slice(qb4 * SB, (qb4 + 1) * SB)
                        nc.tensor.matmul(av_ps[:, sl],
                                         vh[:, kc, 65 * u:65 * u + 65],
                                         expT[:, sl], start=(kc == 0),
                                         stop=(kc == NQT - 1))
                avout = work.tile([128, S], F32, tag="avout", name="avout")
                nc.vector.tensor_copy(avout[:65, :], av_ps[:])
                rec_row = work.tile([1, S], F32, tag="rec_row", name="rec_row")
                nc.vector.reciprocal(rec_row[:], avout[64:65, :])
                rec_bc = work.tile([64, S], F32, tag="rec_bc", name="rec_bc")
                nc.gpsimd.partition_broadcast(rec_bc[:], rec_row[:])
                nc.vector.tensor_mul(ao[:, u, :], avout[:64, :], rec_bc[:])

                # Q-major pass: normalized probs out
                for qt in range(NQT):
                    probs = work.tile([128, S], F32, tag="probs", name="probs")
                    rs2 = work.tile([128, 2], F32, tag="rs2", name="rs2")
                    for hk in range(2):
                        ps = ps_sc.tile([128, 1024], F32, tag="ps_sc",
                                        name=f"scq{hk}")
                        for kb in range(2):
                            sl = slice(kb * SB, (kb + 1) * SB)
                            koff = hk * 1024
                            nc.tensor.matmul(
                                ps[:, sl],
                                qh_u[:, qt * 128:(qt + 1) * 128],
                                kh_u[:, koff + kb * SB:koff + (kb + 1) * SB],
                                start=True, stop=False)
                            nc.tensor.matmul(
                                ps[:, sl], id8[:],
                                mq_sb[:, qt, koff + kb * SB:koff + (kb + 1) * SB],
                                start=False, stop=True)
                        nc.scalar.activation(probs[:, hk * 1024:(hk + 1) * 1024],
                                             ps[:],
                                             mybir.ActivationFunctionType.Exp,
                                             accum_out=rs2[:, hk:hk + 1])
                    rstot = work.tile([128, 1], F32, tag="rstot", name="rstot")
                    nc.vector.tensor_add(rstot[:], rs2[:, 0:1], rs2[:, 1:2])
                    rrec = work.tile([128, 1], F32, tag="rrec", name="rrec")
                    nc.vector.reciprocal(rrec[:], rstot[:])
                    nc.vector.tensor_scalar_mul(probs[:], probs[:], rrec[:])
                    nc.sync.dma_start(
                        attn_d[u, qt * 128:(qt + 1) * 128, :], probs[:])

            # ---- output projection (all 4 heads accumulated) ----
            for qc in range(NQT):
                ps = ps_sc.tile([128, 1024], F32, tag="ps_sc", name="pso")
                for db in range(2):
                    sl = slice(db * SB, (db + 1) * SB)
                    for u in range(HPC):
                        nc.tensor.matmul(ps[:, sl],
                                         ao[:, u, qc * 128:(qc + 1) * 128],
                                         wo_sb[:, u, sl], start=(u == 0),
                                         stop=(u == HPC - 1))
                o_sb = work.tile([128, 1024], F32, tag="o_sb", name="o_sb")
                nc.vector.tensor_copy(o_sb[:], ps[:])
                nc.sync.dma_start(out_d[qc * 128:(qc + 1) * 128, :], o_sb[:])

    nc.compile()
    return nc


_NC_CACHE = {}


def get_nc(reps: int = 1):
    if reps not in _NC_CACHE:
        _NC_CACHE[reps] = build_nc(reps)
    return _NC_CACHE[reps]


def make_in_maps(q, k, v, attn_mask, Wq, Wk, Wv, Wo):
    q = np.asarray(q, np.float32)
    k = np.asarray(k, np.float32)
    v = np.asarray(v, np.float32)
    Wq, Wk, Wv, Wo = (np.asarray(w, np.float32) for w in (Wq, Wk, Wv, Wo))
    mask = np.asarray(attn_mask).reshape(S, S)

    mb = np.where(mask == 0, np.float32(MASK_BIAS), np.float32(0.0))
    mq8 = mb.astype(ml_dtypes.float8_e5m2)
    mt8 = np.ascontiguousarray(mb.T).astype(ml_dtypes.float8_e5m2)

    qT = [np.ascontiguousarray(q[b].T) for b in range(B)]
    kT = [np.ascontiguousarray(k[b].T) for b in range(B)]
    vT = [np.ascontiguousarray(v[b].T) for b in range(B)]

    in_maps = []
    for c in range(NCORES):
        b = c // 4
        heads = [HPC * (c % 4) + j for j in range(HPC)]
        wq = np.stack([
            np.concatenate([Wq[DH * h:DH * (h + 1), :].T for h in heads[2 * p:2 * p + 2]], axis=1)
            for p in range(2)], axis=1)               # [D, 2, 128]
        wk = np.stack([
            np.concatenate([Wk[DH * h:DH * (h + 1), :].T for h in heads[2 * p:2 * p + 2]], axis=1)
            for p in range(2)], axis=1)
        wv = np.concatenate([Wv[DH * h:DH * (h + 1), :].T for h in heads], axis=1)  # [D, 256]
        wo = np.stack([Wo[:, DH * h:DH * (h + 1)].T for h in heads], axis=0)        # [4, 64, D]
        wo = np.ascontiguousarray(wo.transpose(1, 0, 2))                            # [64, 4, D]
        in_maps.append({
            "qT": qT[b], "kT": kT[b], "vT": vT[b],
            "wq": np.ascontiguousarray(wq), "wk": np.ascontiguousarray(wk),
            "wv": np.ascontiguousarray(wv), "wo": wo,
            "mq8": mq8, "mt8": mt8,
        })
    return in_maps


def gather(results):
    attn = np.empty((B, H, S, S), np.float32)
    out = np.zeros((B, S, D), np.float32)
    for c in range(NCORES):
        b = c // 4
        heads = [HPC * (c % 4) + j for j in range(HPC)]
        ap = results[c]["attn_part"]
        for j, h in enumerate(heads):
            attn[b, h] = ap[j]
        out[b] += results[c]["out_part"]
    return out, attn


def kernel(q, k, v, attn_mask, Wq, Wk, Wv, Wo):
    nc = get_nc()
    in_maps = make_in_maps(q, k, v, attn_mask, Wq, Wk, Wv, Wo)
    res = run_bass_kernel_spmd(nc, in_maps, core_ids=list(range(NCORES)))
    return gather(res.results)


# revision 11
# speedup vs baseline: 3.0466x; 3.0466x over previous
"""Trainium2 Bass kernel for nn_MultiHeadAttention (B=2, S=2048, D=1024, H=16).

Sharding: batch x head-groups across 8 cores. Core c handles batch c//4 and
heads 4*(c%4) .. 4*(c%4)+3 (4 "units" per core). Each core computes, per unit:
  - projections qhT/khT (f32r, head-dim on partitions) and vh (bf16, keys on
    partitions) for its 4 heads
  - a k-major pass: scoresT = kh^T q (f32r) + mask bias (fp8 identity-matmul
    add), exp on ScalarE -> probsT (bf16, unnormalized), AV matmul with a
    ones-column appended to vh so row 64 of the PSUM accumulator is the
    per-query masked rowsum; att_out is normalized with 1/rowsum broadcast
  - a q-major pass: scores = qh^T k + mask bias, exp with accum_out giving the
    masked rowsum per query row, normalize on VectorE, DMA the fp32 attention
    probabilities straight out
  - an output projection accumulating all 4 heads in PSUM -> partial out
Host side only reshapes/slices inputs, and gathers: attn slabs are stacked,
partial outs summed across the 4 cores of each batch.
"""

import numpy as np
import ml_dtypes
from contextlib import ExitStack

import concourse.bass as bass
import concourse.tile as tile
from concourse import bacc, mybir
from concourse.bass_utils import run_bass_kernel_spmd
from concourse.masks import make_identity

B, S, D, H = 2, 2048, 1024, 16
DH = D // H          # 64
NCORES = 8
HPC = 4              # heads per core
SB = 512             # matmul moving-dim block
NQT = S // 128       # 16 q/k tiles per unit
NDC = D // 128       # 8 contraction chunks
MASK_BIAS = -32.0    # exp(-32+s) ~ 1e-14: numerically zero vs rowsum ~1e3

F32 = mybir.dt.float32
F32R = mybir.dt.float32r
BF16 = mybir.dt.bfloat16
FP8 = mybir.dt.float8e5


def build_nc(reps: int = 1, bench: bool = False):
    """Build + compile the per-core Bass kernel. reps>1 repeats the whole
    compute (same outputs); bench=True makes the big outputs Internal DRAM
    (identical device work, no host I/O) for wall-clock benchmarking."""
    nc = bacc.Bacc("TRN2", target_bir_lowering=False, debug=False)

    def din(name, shape, dt):
        return nc.dram_tensor(name, shape, dt, kind="ExternalInput").ap()

    def dout(name, shape, dt):
        kind = "Internal" if bench else "ExternalOutput"
        return nc.dram_tensor(name, shape, dt, kind=kind).ap()

    qT_d = din("qT", [D, S], F32)
    kT_d = din("kT", [D, S], F32)
    vT_d = din("vT", [D, S], F32)
    wq_d = din("wq", [D, 2, 128], F32)      # [d, pair, 2*64] head-pair lhsT cols
    wk_d = din("wk", [D, 2, 128], F32)
    wv_d = din("wv", [D, HPC * DH], F32)    # [d, 4*64]
    wo_d = din("wo", [64, HPC, D], F32)     # [dh, unit, dout]
    mq_d = din("mq8", [S, S], FP8)          # q-major mask bias (0 / -32)
    mt_d = din("mt8", [S, S], FP8)          # k-major (transposed) mask bias
    attn_d = dout("attn_part", [HPC, S, S], F32)
    out_d = dout("out_part", [S, D], F32)
    if bench:
        # keep one tiny real output so the graph isn't dead
        flag_d = nc.dram_tensor("flag", [1, 1], F32, kind="ExternalOutput").ap()

    with tile.TileContext(nc) as tc, ExitStack() as ctx:
        res = ctx.enter_context(tc.tile_pool(name="res", bufs=1))
        xin = ctx.enter_context(tc.tile_pool(name="xin", bufs=3))
        xr = ctx.enter_context(tc.tile_pool(name="xr", bufs=2))
        work = ctx.enter_context(tc.tile_pool(name="work", bufs=2))
        ps_sc = ctx.enter_context(tc.tile_pool(name="ps_sc", bufs=2, space="PSUM"))
        ps_av = ctx.enter_context(tc.tile_pool(name="ps_av", bufs=1, space="PSUM"))

        # ---- resident tensors ----
        mq_sb = res.tile([128, NQT, S], FP8)     # [p, qtile, k]
        nc.sync.dma_start(mq_sb[:], mq_d.rearrange("(t p) k -> p t k", p=128))
        mt_sb = res.tile([128, NQT, S], FP8)     # [p, ktile, q]
        nc.sync.dma_start(mt_sb[:], mt_d.rearrange("(t p) q -> p t q", p=128))

        id_f = res.tile([128, 128], F32)
        make_identity(nc, id_f[:])
        id8 = res.tile([128, 128], FP8)
        nc.vector.tensor_copy(id8[:], id_f[:])

        def load_round(dram_ap, shape, dt, dst_ap, name):
            # stage through a slot shared with the (later-lived) probs tag
            t_f = work.tile([128, S], F32, tag="probs", name=name + "_f")
            flat = int(np.prod(shape[1:]))
            npart = shape[0]
            dims = {chr(98 + i): shape[2 + i] for i in range(len(shape) - 2)}
            pat = "p (" + " ".join(["a"] + sorted(dims)) + ") -> p " + " ".join(["a"] + sorted(dims))
            t_fs = t_f[:npart, :flat].rearrange(pat, **dims)
            nc.sync.dma_start(t_fs, dram_ap)
            nc.vector.tensor_copy(dst_ap, t_fs)

        wq_sb = res.tile([128, NDC, 2, 128], F32R, name="wq_sb")
        load_round(wq_d.rearrange("(c p) j m -> p c j m", p=128),
                   [128, NDC, 2, 128], F32R, wq_sb[:], "wq")
        wk_sb = res.tile([128, NDC, 2, 128], F32R, name="wk_sb")
        load_round(wk_d.rearrange("(c p) j m -> p c j m", p=128),
                   [128, NDC, 2, 128], F32R, wk_sb[:], "wk")
        wv_sb = res.tile([128, NDC, HPC * DH], BF16, name="wv_sb")
        load_round(wv_d.rearrange("(c p) m -> p c m", p=128),
                   [128, NDC, HPC * DH], BF16, wv_sb[:], "wv")
        wo_sb = res.tile([64, HPC, D], BF16, name="wo_sb")
        for wh in range(2):
            load_round(wo_d[:, 2 * wh:2 * wh + 2, :], [64, 2, D], BF16,
                       wo_sb[:, 2 * wh:2 * wh + 2, :], f"wo{wh}")

        qhT = res.tile([128, 2, S], F32R)        # [64*half+dh, pair, s]
        khT = res.tile([128, 2, S], F32R)
        vh = res.tile([128, NQT, HPC * 128], BF16)  # [key, kchunk, 128h + (dh|ones)]
        ao = res.tile([64, HPC, S], BF16)        # normalized att_out^T per unit

        nc.gpsimd.memset(
            vh[:].rearrange("p c (h x) -> p c h x", x=128)[:, :, :, 64:128], 1.0)

        for _ in range(reps):
            # ---- projections: qhT / khT ----
            for src_d, wsb, dst in ((qT_d, wq_sb, qhT), (kT_d, wk_sb, khT)):
                for sb_i in range(S // SB):
                    ps = ps_sc.tile([128, 1024], F32, tag="ps_sc", name="pspr")
                    for c in range(NDC):
                        x_f = xin.tile([128, SB], F32, tag="x_f", name="x_f")
                        nc.sync.dma_start(
                            x_f[:], src_d[c * 128:(c + 1) * 128,
                                          sb_i * SB:(sb_i + 1) * SB])
                        x_r = xr.tile([128, SB], F32R, tag="x_r", name="x_r")
                        nc.vector.tensor_copy(x_r[:], x_f[:])
                        for pair in range(2):
                            nc.tensor.matmul(ps[:, pair * SB:(pair + 1) * SB],
                                             wsb[:, c, pair, :],
                                             x_r[:], start=(c == 0),
                                             stop=(c == NDC - 1))
                    nc.vector.tensor_copy(
                        dst[:, :, sb_i * SB:(sb_i + 1) * SB],
                        ps[:].rearrange("p (j s) -> p j s", j=2))

            # ---- projections: vh (keys on partitions, bf16) ----
            for sb_i in range(S // SB):
                # one 256-wide accumulation region per PSUM bank: start=True
                # clears the whole bank, so regions must be bank-exclusive
                ps = ps_av.tile([128, S], F32, tag="ps_av", name="psv")
                for c in range(NDC):
                    x_f = xin.tile([128, SB], F32, tag="x_f", name="xv_f")
                    nc.sync.dma_start(
                        x_f[:], vT_d[c * 128:(c + 1) * 128,
                                     sb_i * SB:(sb_i + 1) * SB])
                    x_b = xr.tile([128, SB], BF16, tag="x_b", name="xv_b")
                    nc.vector.tensor_copy(x_b[:], x_f[:])
                    for j in range(SB // 128):
                        nc.tensor.matmul(ps[:, j * 512:j * 512 + HPC * DH],
                                         x_b[:, j * 128:(j + 1) * 128],
                                         wv_sb[:, c, :], start=(c == 0),
                                         stop=(c == NDC - 1))
                for j in range(SB // 128):
                    kc = sb_i * (SB // 128) + j
                    nc.vector.tensor_copy(
                        vh[:, kc, :].rearrange("p (h x) -> p h x", x=128)[:, :, :64],
                        ps[:, j * 512:j * 512 + HPC * DH]
                        .rearrange("p (h x) -> p h x", x=64))

            # ---- per-unit attention ----
            for u in range(HPC):
                half, pair = u % 2, u // 2
                qh_u = qhT[64 * half:64 * half + 64, pair, :]
                kh_u = khT[64 * half:64 * half + 64, pair, :]

                # K-major pass: AV + rowsums
                av_ps = ps_av.tile([128, S], F32, tag="ps_av", name="av_ps")
                for kc in range(NQT):
                    sc_ps = []
                    for hk in range(2):
                        ps = ps_sc.tile([128, 1024], F32, tag="ps_sc",
                                        name=f"sct{hk}")
                        for qb in range(2):
                            sl = slice(qb * SB, (qb + 1) * SB)
                            qoff = hk * 1024
                            nc.tensor.matmul(
                                ps[:, sl],
                                kh_u[:, kc * 128:(kc + 1) * 128],
                                qh_u[:, qoff + qb * SB:qoff + (qb + 1) * SB],
                                start=True, stop=False)
                            nc.tensor.matmul(
                                ps[:, sl], id8[:],
                                mt_sb[:, kc, qoff + qb * SB:qoff + (qb + 1) * SB],
                                start=False, stop=True)
                        sc_ps.append(ps)
                    for hk in range(2):
                        expT = work.tile([128, 1024], BF16, tag="expT",
                                         name="expT")
                        nc.scalar.activation(expT[:],
                                             sc_ps[hk][:],
                                             mybir.ActivationFunctionType.Exp)
                        for qb in range(2):
                            sl = slice(hk * 1024 + qb * SB,
                                       hk * 1024 + (qb + 1) * SB)
                            nc.tensor.matmul(av_ps[:, sl],
                                             vh[:, kc, 128 * u:128 * u + 128],
                                             expT[:, qb * SB:(qb + 1) * SB],
                                             start=(kc == 0),
                                             stop=(kc == NQT - 1))
                rec_bc = work.tile([64, S], F32, tag="rec_bc", name="rec_bc", bufs=1)
                nc.vector.reciprocal(rec_bc[:], av_ps[64:128, :])
                nc.vector.tensor_mul(ao[:, u, :], av_ps[:64, :], rec_bc[:])

                # Q-major pass: normalized probs out
                for qt in range(NQT):
                    probs = work.tile([128, S], F32, tag="probs", name="probs")
                    rs2 = work.tile([128, 2], F32, tag="rs2", name="rs2")
                    for hk in range(2):
                        ps = ps_sc.tile([128, 1024], F32, tag="ps_sc",
                                        name=f"scq{hk}")
                        for kb in range(2):
                            sl = slice(kb * SB, (kb + 1) * SB)
                            koff = hk * 1024
                            nc.tensor.matmul(
                                ps[:, sl],
                                qh_u[:, qt * 128:(qt + 1) * 128],
                                kh_u[:, koff + kb * SB:koff + (kb + 1) * SB],
                                start=True, stop=False)
                            nc.tensor.matmul(
                                ps[:, sl], id8[:],
                                mq_sb[:, qt, koff + kb * SB:koff + (kb + 1) * SB],
                                start=False, stop=True)
                        nc.scalar.activation(probs[:, hk * 1024:(hk + 1) * 1024],
                                             ps[:],
                                             mybir.ActivationFunctionType.Exp,
                                             accum_out=rs2[:, hk:hk + 1])
                    rstot = work.tile([128, 1], F32, tag="rstot", name="rstot")
                    nc.vector.tensor_add(rstot[:], rs2[:, 0:1], rs2[:, 1:2])
                    rrec = work.tile([128, 1], F32, tag="rrec", name="rrec")
                    nc.vector.reciprocal(rrec[:], rstot[:])
                    nc.vector.tensor_scalar_mul(probs[:], probs[:], rrec[:])
                    nc.sync.dma_start(
                        attn_d[u, qt * 128:(qt + 1) * 128, :], probs[:])

            # ---- output projection (all 4 heads accumulated) ----
            for qc in range(NQT):
                ps = ps_sc.tile([128, 1024], F32, tag="ps_sc", name="pso")
                for db in range(2):
                    sl = slice(db * SB, (db + 1) * SB)
                    for u in range(HPC):
                        nc.tensor.matmul(ps[:, sl],
                                         ao[:, u, qc * 128:(qc + 1) * 128],
                                         wo_sb[:, u, sl], start=(u == 0),
                                         stop=(u == HPC - 1))
                for db in range(2):
                    o_sb = work.tile([128, SB], F32, tag="o_sb", name="o_sb",
                                     bufs=1)
                    nc.vector.tensor_copy(o_sb[:], ps[:, db * SB:(db + 1) * SB])
                    nc.sync.dma_start(
                        out_d[qc * 128:(qc + 1) * 128, db * SB:(db + 1) * SB],
                        o_sb[:])

        if bench:
            nc.sync.dma_start(flag_d[:], o_sb[0:1, 0:1])

    nc.compile()
    return nc


_NC_CACHE = {}


def get_nc(reps: int = 1):
    if reps not in _NC_CACHE:
        _NC_CACHE[reps] = build_nc(reps)
    return _NC_CACHE[reps]


def make_in_maps(q, k, v, attn_mask, Wq, Wk, Wv, Wo):
    q = np.asarray(q, np.float32)
    k = np.asarray(k, np.float32)
    v = np.asarray(v, np.float32)
    Wq, Wk, Wv, Wo = (np.asarray(w, np.float32) for w in (Wq, Wk, Wv, Wo))
    mask = np.asarray(attn_mask).reshape(S, S)

    mb = np.where(mask == 0, np.float32(MASK_BIAS), np.float32(0.0))
    mq8 = mb.astype(ml_dtypes.float8_e5m2)
    mt8 = np.ascontiguousarray(mb.T).astype(ml_dtypes.float8_e5m2)

    qT = [np.ascontiguousarray(q[b].T) for b in range(B)]
    kT = [np.ascontiguousarray(k[b].T) for b in range(B)]
    vT = [np.ascontiguousarray(v[b].T) for b in range(B)]

    in_maps = []
    for c in range(NCORES):
        b = c // 4
        heads = [HPC * (c % 4) + j for j in range(HPC)]
        wq = np.stack([
            np.concatenate([Wq[DH * h:DH * (h + 1), :].T for h in heads[2 * p:2 * p + 2]], axis=1)
            for p in range(2)], axis=1) / np.sqrt(DH)  # [D, 2, 128]; 1/sqrt(dh) folded in
        wk = np.stack([
            np.concatenate([Wk[DH * h:DH * (h + 1), :].T for h in heads[2 * p:2 * p + 2]], axis=1)
            for p in range(2)], axis=1)
        wv = np.concatenate([Wv[DH * h:DH * (h + 1), :].T for h in heads], axis=1)  # [D, 256]
        wo = np.stack([Wo[:, DH * h:DH * (h + 1)].T for h in heads], axis=0)        # [4, 64, D]
        wo = np.ascontiguousarray(wo.transpose(1, 0, 2))                            # [64, 4, D]
        in_maps.append({
            "qT": qT[b], "kT": kT[b], "vT": vT[b],
            "wq": np.ascontiguousarray(wq), "wk": np.ascontiguousarray(wk),
            "wv": np.ascontiguousarray(wv), "wo": wo,
            "mq8": mq8, "mt8": mt8,
        })
    return in_maps


def gather(results):
    attn = np.empty((B, H, S, S), np.float32)
    out = np.zeros((B, S, D), np.float32)
    for c in range(NCORES):
        b = c // 4
        heads = [HPC * (c % 4) + j for j in range(HPC)]
        ap = results[c]["attn_part"]
        for j, h in enumerate(heads):
            attn[b, h] = ap[j]
        out[b] += results[c]["out_part"]
    return out, attn


def kernel(q, k, v, attn_mask, Wq, Wk, Wv, Wo):
    nc = get_nc()
    in_maps = make_in_maps(q, k, v, attn_mask, Wq, Wk, Wv, Wo)
    res = run_bass_kernel_spmd(nc, in_maps, core_ids=list(range(NCORES)))
    return gather(res.results)
